# revision 8
# baseline (speedup 1.0000x reference)
"""Trainium2 Bass kernel for per-token multi-head self-attention.

Computation (per token t):
  q,k,v = x @ W{q,k,v}.T ; scores = (q_t k_t^T)/sqrt(128) over heads [16x16]
  out_t = softmax(scores) @ v_t ; y = out @ Wo.T

Sharding: data-parallel over the 16384 tokens -> 8 cores x 2048 tokens.
All activations flow on-chip in transposed ([feature, token]) layout; the
host pre-transposes x shards and weights so every matmul operand loads
naturally with the contraction dim on partitions (no on-chip transposes for
the 4 big matmuls). fp32r (full-rate tf32-like) for the big matmuls.

Middle stage per 4-token group: per-token 16x16 score matmuls -> exp (ACT)
into a block-diagonal [128,64] attn matrix -> one AV matmul against the
PE-transposed [4tok x 32, d] V block (with a ones column producing the
softmax normalizer Z) -> per-partition 1/Z scale -> PE-transpose back.
"""
import math
from contextlib import ExitStack

import numpy as np

NCORES = 8
E = 2048          # hidden
NH = 16           # heads
HD = 128          # head dim
TPC = 2048        # tokens per core
TC = 512          # token chunk in pass B
P = 128

_cached = {}


def _build_program():
    import concourse.bass as bass
    import concourse.tile as tile
    from concourse import bacc, mybir
    from concourse.masks import make_identity

    f32 = mybir.dt.float32
    f32r = mybir.dt.float32r

    nc = bacc.Bacc("TRN2", target_bir_lowering=False, debug=False)

    xT_d = nc.dram_tensor("xT", [E, TPC], f32r, kind="ExternalInput").ap()
    WqT_d = nc.dram_tensor("WqT", [E, E], f32r, kind="ExternalInput").ap()
    WkT_d = nc.dram_tensor("WkT", [E, E], f32r, kind="ExternalInput").ap()
    WvT_d = nc.dram_tensor("WvT", [E, E], f32r, kind="ExternalInput").ap()
    WoT_d = nc.dram_tensor("WoT", [E, E], f32r, kind="ExternalInput").ap()
    yT_d = nc.dram_tensor("yT", [E, TPC], f32, kind="ExternalOutput").ap()

    qT_d = nc.dram_tensor("qT_scr", [E, TPC], f32).ap()
    kT_d = nc.dram_tensor("kT_scr", [E, TPC], f32).ap()
    vT_d = nc.dram_tensor("vT_scr", [E, TPC], f32).ap()

    NE = E // P   # 16 k-tiles
    NO = E // P   # 16 o-tiles
    SC = 1.0 / math.sqrt(HD)

    with tile.TileContext(nc) as tc, ExitStack() as ctx:
        glob = ctx.enter_context(tc.tile_pool(name="glob", bufs=1))
        ident = glob.tile([P, P], f32)
        make_identity(nc, ident)

        # ============ PASS A: qT/kT/vT = (W @ x.T) -> DRAM ============
        with tc.tile_pool(name="xsb", bufs=1) as xpool, \
             tc.tile_pool(name="wA", bufs=6) as wpool, \
             tc.tile_pool(name="psA", bufs=8, space="PSUM") as pspool, \
             tc.tile_pool(name="stA", bufs=4) as stpool:
            xsb = xpool.tile([P, NE, TPC], f32r)
            for e in range(NE):
                nc.sync.dma_start(out=xsb[:, e, :], in_=xT_d[e * P:(e + 1) * P, :])

            wmats = [WqT_d, WkT_d, WvT_d]
            outs = [qT_d, kT_d, vT_d]
            for oi in range(NO):
                wg = []
                for m in range(3):
                    wt = wpool.tile([P, NE, P], f32r, tag="wA")
                    for e in range(NE):
                        nc.sync.dma_start(
                            out=wt[:, e, :],
                            in_=wmats[m][e * P:(e + 1) * P,
                                         oi * P:(oi + 1) * P])
                    wg.append(wt)
                for tcix in range(TPC // TC):
                    for m in range(3):
                        acc = pspool.tile([P, TC], f32, tag="accA")
                        for e in range(NE):
                            nc.tensor.matmul(
                                acc,
                                wg[m][:, e, :],
                                xsb[:, e, tcix * TC:(tcix + 1) * TC],
                                start=(e == 0), stop=(e == NE - 1))
                        st = stpool.tile([P, TC], f32, tag="stA")
                        nc.vector.tensor_copy(st, acc)
                        nc.sync.dma_start(
                            out=outs[m][oi * P:(oi + 1) * P,
                                        tcix * TC:(tcix + 1) * TC],
                            in_=st)

        # ============ PASS B: attention + Wo ============
        NG = TC // 4           # 4-token groups per chunk
        SUB = 64               # tokens per v2 relayout block
        with tc.tile_pool(name="qkv", bufs=1) as qkvp, \
             tc.tile_pool(name="v2p", bufs=1) as v2p, \
             tc.tile_pool(name="bdp", bufs=1) as bdp, \
             tc.tile_pool(name="vgp", bufs=1) as vgp, \
             tc.tile_pool(name="mid", bufs=4) as mid, \
             tc.tile_pool(name="aop", bufs=2) as aop, \
             tc.tile_pool(name="woP", bufs=2) as woP, \
             tc.tile_pool(name="yst", bufs=3) as yst, \
             tc.tile_pool(name="psS", bufs=2, space="PSUM") as psS, \
             tc.tile_pool(name="psM", bufs=4, space="PSUM") as psM, \
             tc.tile_pool(name="psY", bufs=2, space="PSUM") as psY:

            # persistent manually-rotated slots (stable zero padding)
            NBD = 8
            bd_slots = []
            for i in range(NBD):
                t = bdp.tile([P, 64], f32, tag=f"bd{i}")
                nc.vector.memset(t, 0.0)
                bd_slots.append(t)
            NV2 = 2
            v2_slots = []
            for i in range(NV2):
                t = v2p.tile([P, SUB, 32], f32, tag=f"v2_{i}")
                nc.vector.memset(t, 0.0)
                v2_slots.append(t)
            NVG = 8
            vg_slots = []
            for i in range(NVG):
                t = vgp.tile([P, HD + 1], f32, tag=f"vg{i}")
                nc.vector.memset(t[:, HD:HD + 1], 1.0)
                vg_slots.append(t)

            gi_all = 0
            v2i = 0
            for tcix in range(TPC // TC):
                t0 = tcix * TC
                q_sb = qkvp.tile([P, NH, TC], f32, tag="q")
                k_sb = qkvp.tile([P, NH, TC], f32, tag="k")
                v_sb = qkvp.tile([P, NH, TC], f32, tag="v")
                for g in range(NH):
                    nc.sync.dma_start(out=q_sb[:, g, :],
                                      in_=qT_d[g * P:(g + 1) * P, t0:t0 + TC])
                    nc.sync.dma_start(out=k_sb[:, g, :],
                                      in_=kT_d[g * P:(g + 1) * P, t0:t0 + TC])
                    nc.sync.dma_start(out=v_sb[:, g, :],
                                      in_=vT_d[g * P:(g + 1) * P, t0:t0 + TC])

                aoT = aop.tile([P, NH, TC], f32r, tag="aoT")

                for sub in range(TC // SUB):
                    # relayout v to token-major with padded 32-col slots
                    v2 = v2_slots[v2i % NV2]
                    v2i += 1
                    nc.vector.tensor_copy(
                        v2[:, :, 0:NH],
                        v_sb[:, :, sub * SUB:(sub + 1) * SUB]
                        .rearrange("p g t -> p t g"))

                    for gi4 in range(SUB // 4):
                        tt = sub * SUB + gi4 * 4   # first token in group
                        bd = bd_slots[gi_all % NBD]
                        vg = vg_slots[gi_all % NVG]
                        gi_all += 1

                        # V block transpose: [128, 4*32] -> [4*32, 128]
                        vg_ps = psM.tile([P, P], f32, tag="mps")
                        nc.tensor.transpose(
                            vg_ps,
                            v2[:, gi4 * 4:(gi4 + 1) * 4, :]
                            .rearrange("p t g -> p (t g)"),
                            ident)
                        nc.vector.tensor_copy(vg[:, 0:HD], vg_ps)

                        # scores for 4 tokens -> one psum tile at 32-strips
                        sc_ps = psS.tile([P, NH], f32, tag="scps")
                        for j in range(4):
                            t = tt + j
                            nc.tensor.matmul(
                                sc_ps[32 * j:32 * j + NH, :],
                                k_sb[:, :, t], q_sb[:, :, t],
                                start=True, stop=True,
                                tile_position=(0, 32 * j))
                        # exp into block-diagonal [g,h] blocks
                        for j in range(4):
                            nc.scalar.activation(
                                out=bd[32 * j:32 * j + NH,
                                       NH * j:NH * (j + 1)],
                                in_=sc_ps[32 * j:32 * j + NH, :],
                                func=mybir.ActivationFunctionType.Exp,
                                scale=SC)

                        # AV: [64,(t,h)] x [128, d+1]
                        av_ps = psM.tile([P, HD + 1], f32, tag="mps")
                        nc.tensor.matmul(av_ps[0:64, :], bd, vg, start=True, stop=True)

                        invz = mid.tile([64, 1], f32, tag="invz")
                        nc.vector.reciprocal(invz, av_ps[0:64, HD:HD + 1])
                        ao = mid.tile([64, HD], f32, tag="ao")
                        nc.vector.tensor_scalar_mul(ao, av_ps[0:64, 0:HD], invz)

                        # transpose back: [64,(t,h) x 128 d] -> [128 d, 64]
                        aoT_ps = psM.tile([P, 64], f32, tag="mps")
                        nc.tensor.transpose(aoT_ps, ao, ident[0:64, 0:64])
                        nc.vector.tensor_copy(
                            aoT[:, :, tt:tt + 4].rearrange("p h t -> p h t"),
                            aoT_ps.rearrange("p (t h) -> p h t", t=4))

                # ---- y.T = Wo @ attnOut.T for this chunk ----
                for oi in range(NO):
                    wo = woP.tile([P, NH, P], f32r, tag="wo")
                    nc.sync.dma_start(
                        out=wo,
                        in_=WoT_d[:, oi * P:(oi + 1) * P]
                        .rearrange("(hh p) o -> p hh o", p=P))
                    yp = psY.tile([P, TC], f32, tag="yps")
                    for h in range(NH):
                        nc.tensor.matmul(
                            yp, wo[:, h, :], aoT[:, h, :],
                            start=(h == 0), stop=(h == NH - 1))
                    ys = yst.tile([P, TC], f32, tag="ys")
                    nc.vector.tensor_copy(ys, yp)
                    nc.sync.dma_start(
                        out=yT_d[oi * P:(oi + 1) * P, t0:t0 + TC], in_=ys)

    nc.compile()
    return nc


def _get_program():
    if "nc" not in _cached:
        _cached["nc"] = _build_program()
    return _cached["nc"]


def kernel(x, Wq, Wk, Wv, Wo):
    from concourse.bass_utils import run_bass_kernel_spmd

    B, S, H = x.shape
    assert (B * S, H) == (NCORES * TPC, E)
    nc = _get_program()

    xf = np.ascontiguousarray(x.reshape(B * S, H))
    WqT = np.ascontiguousarray(Wq.T)
    WkT = np.ascontiguousarray(Wk.T)
    WvT = np.ascontiguousarray(Wv.T)
    WoT = np.ascontiguousarray(Wo.T)

    in_maps = []
    for i in range(NCORES):
        xT = np.ascontiguousarray(xf[i * TPC:(i + 1) * TPC, :].T)
        in_maps.append({"xT": xT, "WqT": WqT, "WkT": WkT,
                        "WvT": WvT, "WoT": WoT})

    import os
    trace = bool(int(os.environ.get("BASS_KERNEL_TRACE", "0")))
    res = run_bass_kernel_spmd(nc, in_maps, core_ids=list(range(NCORES)),
                               trace=trace)
    if trace:
        _cached["last_results"] = res
    parts = [res.results[i]["yT"].T for i in range(NCORES)]
    y = np.concatenate(parts, axis=0).reshape(B, S, H)
    return np.ascontiguousarray(y.astype(np.float32))


# revision 10
# speedup vs baseline: 1.0964x; 1.0964x over previous
"""Trainium2 Bass kernel for per-token multi-head self-attention.

Computation (per token t):
  q,k,v = x @ W{q,k,v}.T ; scores = (q_t k_t^T)/sqrt(128) over heads [16x16]
  out_t = softmax(scores) @ v_t ; y = out @ Wo.T

Sharding: data-parallel over the 16384 tokens -> 8 cores x 2048 tokens.
All activations flow on-chip in transposed ([feature, token]) layout; the
host pre-transposes x shards and weights so every matmul operand loads
naturally with the contraction dim on partitions (no on-chip transposes for
the 4 big matmuls). fp32r (full-rate tf32-like) for the big matmuls.

Middle stage per 4-token group: per-token 16x16 score matmuls -> exp (ACT)
into a block-diagonal [128,64] attn matrix -> one AV matmul against the
PE-transposed [4tok x 32, d] V block (with a ones column producing the
softmax normalizer Z) -> per-partition 1/Z scale -> PE-transpose back.
"""
import math
from contextlib import ExitStack

import numpy as np

NCORES = 8
E = 2048          # hidden
NH = 16           # heads
HD = 128          # head dim
TPC = 2048        # tokens per core
TC = 512          # token chunk in pass B
P = 128

_cached = {}


def _build_program():
    import concourse.bass as bass
    import concourse.tile as tile
    from concourse import bacc, mybir
    from concourse.masks import make_identity

    f32 = mybir.dt.float32
    f32r = mybir.dt.float32r

    nc = bacc.Bacc("TRN2", target_bir_lowering=False, debug=False)

    xT_d = nc.dram_tensor("xT", [E, TPC], f32r, kind="ExternalInput").ap()
    WqT_d = nc.dram_tensor("WqT", [E, E], f32r, kind="ExternalInput").ap()
    WkT_d = nc.dram_tensor("WkT", [E, E], f32r, kind="ExternalInput").ap()
    WvT_d = nc.dram_tensor("WvT", [E, E], f32r, kind="ExternalInput").ap()
    WoT_d = nc.dram_tensor("WoT", [E, E], f32r, kind="ExternalInput").ap()
    yT_d = nc.dram_tensor("yT", [E, TPC], f32, kind="ExternalOutput").ap()

    qT_d = nc.dram_tensor("qT_scr", [E, TPC], f32).ap()
    kT_d = nc.dram_tensor("kT_scr", [E, TPC], f32).ap()
    vT_d = nc.dram_tensor("vT_scr", [E, TPC], f32).ap()

    NE = E // P   # 16 k-tiles
    NO = E // P   # 16 o-tiles
    SC = 1.0 / math.sqrt(HD)

    with tile.TileContext(nc) as tc, ExitStack() as ctx:
        glob = ctx.enter_context(tc.tile_pool(name="glob", bufs=1))
        ident = glob.tile([P, P], f32)
        make_identity(nc, ident)

        # ============ PASS A: qT/kT/vT = (W @ x.T) -> DRAM ============
        with tc.tile_pool(name="xsb", bufs=1) as xpool, \
             tc.tile_pool(name="wA", bufs=6) as wpool, \
             tc.tile_pool(name="psA", bufs=8, space="PSUM") as pspool, \
             tc.tile_pool(name="stA", bufs=4) as stpool:
            xsb = xpool.tile([P, NE, TPC], f32r)
            for e in range(NE):
                nc.sync.dma_start(out=xsb[:, e, :], in_=xT_d[e * P:(e + 1) * P, :])

            wmats = [WqT_d, WkT_d, WvT_d]
            outs = [qT_d, kT_d, vT_d]
            for oi in range(NO):
                wg = []
                for m in range(3):
                    wt = wpool.tile([P, NE, P], f32r, tag="wA")
                    for e in range(NE):
                        nc.sync.dma_start(
                            out=wt[:, e, :],
                            in_=wmats[m][e * P:(e + 1) * P,
                                         oi * P:(oi + 1) * P])
                    wg.append(wt)
                for tcix in range(TPC // TC):
                    for m in range(3):
                        acc = pspool.tile([P, TC], f32, tag="accA")
                        for e in range(NE):
                            nc.tensor.matmul(
                                acc,
                                wg[m][:, e, :],
                                xsb[:, e, tcix * TC:(tcix + 1) * TC],
                                start=(e == 0), stop=(e == NE - 1))
                        st = stpool.tile([P, TC], f32, tag="stA")
                        nc.vector.tensor_copy(st, acc)
                        nc.sync.dma_start(
                            out=outs[m][oi * P:(oi + 1) * P,
                                        tcix * TC:(tcix + 1) * TC],
                            in_=st)

        # ============ PASS B: attention + Wo ============
        NG = TC // 4           # 4-token groups per chunk
        SUB = 64               # tokens per v2 relayout block
        with tc.tile_pool(name="qkv", bufs=1) as qkvp, \
             tc.tile_pool(name="v2p", bufs=1) as v2p, \
             tc.tile_pool(name="bdp", bufs=1) as bdp, \
             tc.tile_pool(name="vgp", bufs=1) as vgp, \
             tc.tile_pool(name="mid", bufs=4) as mid, \
             tc.tile_pool(name="aop", bufs=2) as aop, \
             tc.tile_pool(name="woP", bufs=2) as woP, \
             tc.tile_pool(name="yst", bufs=3) as yst, \
             tc.tile_pool(name="psS", bufs=2, space="PSUM") as psS, \
             tc.tile_pool(name="psM", bufs=4, space="PSUM") as psM, \
             tc.tile_pool(name="psY", bufs=2, space="PSUM") as psY:

            # persistent manually-rotated slots (stable zero padding)
            NBD = 8
            bd_slots = []
            for i in range(NBD):
                t = bdp.tile([P, 64], f32, tag=f"bd{i}")
                nc.vector.memset(t, 0.0)
                bd_slots.append(t)
            NV2 = 2
            v2_slots = []
            for i in range(NV2):
                t = v2p.tile([P, SUB, 32], f32, tag=f"v2_{i}")
                nc.vector.memset(t, 0.0)
                v2_slots.append(t)
            NVG = 8
            vg_slots = []
            for i in range(NVG):
                t = vgp.tile([P, HD + 1], f32, tag=f"vg{i}")
                nc.vector.memset(t[:, HD:HD + 1], 1.0)
                vg_slots.append(t)

            # Wo matmul stream for chunk c-1, interleaved 2 MMs per middle
            # group of chunk c so the PE never idles long enough to cool.
            wo_seq = [(oi, h) for oi in range(NO) for h in range(NH)]

            def wo_step(state, nsteps):
                for _ in range(nsteps):
                    if state is None or state["pos"] >= len(wo_seq):
                        return
                    oi, h = wo_seq[state["pos"]]
                    state["pos"] += 1
                    if h == 0:
                        wo = woP.tile([P, NH, P], f32r, tag="wo", name="wo")
                        nc.sync.dma_start(
                            out=wo,
                            in_=WoT_d[:, oi * P:(oi + 1) * P]
                            .rearrange("(hh p) o -> p hh o", p=P))
                        state["wo"] = wo
                        state["yp"] = psY.tile([P, TC], f32, tag="yps", name="yps")
                    nc.tensor.matmul(
                        state["yp"], state["wo"][:, h, :],
                        state["aoT"][:, h, :],
                        start=(h == 0), stop=(h == NH - 1))
                    if h == NH - 1:
                        ys = yst.tile([P, TC], f32, tag="ys")
                        nc.vector.tensor_copy(ys, state["yp"])
                        nc.sync.dma_start(
                            out=yT_d[oi * P:(oi + 1) * P,
                                     state["t0"]:state["t0"] + TC],
                            in_=ys)

            gi_all = 0
            v2i = 0
            prev = None
            for tcix in range(TPC // TC):
                t0 = tcix * TC
                q_sb = qkvp.tile([P, NH, TC], f32, tag="q")
                k_sb = qkvp.tile([P, NH, TC], f32, tag="k")
                v_sb = qkvp.tile([P, NH, TC], f32, tag="v")
                for g in range(NH):
                    nc.sync.dma_start(out=q_sb[:, g, :],
                                      in_=qT_d[g * P:(g + 1) * P, t0:t0 + TC])
                    nc.sync.dma_start(out=k_sb[:, g, :],
                                      in_=kT_d[g * P:(g + 1) * P, t0:t0 + TC])
                    nc.sync.dma_start(out=v_sb[:, g, :],
                                      in_=vT_d[g * P:(g + 1) * P, t0:t0 + TC])

                aoT = aop.tile([P, NH, TC], f32r, tag="aoT")

                for sub in range(TC // SUB):
                    # relayout v to token-major with padded 32-col slots
                    v2 = v2_slots[v2i % NV2]
                    v2i += 1
                    nc.vector.tensor_copy(
                        v2[:, :, 0:NH],
                        v_sb[:, :, sub * SUB:(sub + 1) * SUB]
                        .rearrange("p g t -> p t g"))

                    for gi4 in range(SUB // 4):
                        tt = sub * SUB + gi4 * 4   # first token in group
                        bd = bd_slots[gi_all % NBD]
                        vg = vg_slots[gi_all % NVG]
                        gi_all += 1

                        # V block transpose: [128, 4*32] -> [4*32, 128]
                        vg_ps = psM.tile([P, P], f32, tag="mps")
                        nc.tensor.transpose(
                            vg_ps,
                            v2[:, gi4 * 4:(gi4 + 1) * 4, :]
                            .rearrange("p t g -> p (t g)"),
                            ident)
                        nc.vector.tensor_copy(vg[:, 0:HD], vg_ps)

                        # scores for 4 tokens -> one psum tile at 32-strips
                        sc_ps = psS.tile([P, NH], f32, tag="scps")
                        for j in range(4):
                            t = tt + j
                            nc.tensor.matmul(
                                sc_ps[32 * j:32 * j + NH, :],
                                k_sb[:, :, t], q_sb[:, :, t],
                                start=True, stop=True,
                                tile_position=(0, 32 * j))
                        # exp into block-diagonal [g,h] blocks
                        for j in range(4):
                            nc.scalar.activation(
                                out=bd[32 * j:32 * j + NH,
                                       NH * j:NH * (j + 1)],
                                in_=sc_ps[32 * j:32 * j + NH, :],
                                func=mybir.ActivationFunctionType.Exp,
                                scale=SC)

                        # AV: [64,(t,h)] x [128, d+1]
                        av_ps = psM.tile([P, HD + 1], f32, tag="mps")
                        nc.tensor.matmul(av_ps[0:64, :], bd, vg, start=True, stop=True)

                        invz = mid.tile([64, 1], f32, tag="invz")
                        nc.vector.reciprocal(invz, av_ps[0:64, HD:HD + 1])
                        ao = mid.tile([64, HD], f32, tag="ao")
                        nc.vector.tensor_scalar_mul(ao, av_ps[0:64, 0:HD], invz)

                        # transpose back: [64,(t,h) x 128 d] -> [128 d, 64]
                        aoT_ps = psM.tile([P, 64], f32, tag="mps")
                        nc.tensor.transpose(aoT_ps, ao, ident[0:64, 0:64])
                        nc.vector.tensor_copy(
                            aoT[:, :, tt:tt + 4].rearrange("p h t -> p h t"),
                            aoT_ps.rearrange("p (t h) -> p h t", t=4))
                        wo_step(prev, 2)

                # drain any remainder of the previous chunk's Wo stream
                wo_step(prev, len(wo_seq))
                prev = {"pos": 0, "aoT": aoT, "t0": t0, "wo": None, "yp": None}
            wo_step(prev, len(wo_seq))

    nc.compile()
    return nc


def _get_program():
    if "nc" not in _cached:
        _cached["nc"] = _build_program()
    return _cached["nc"]


def kernel(x, Wq, Wk, Wv, Wo):
    from concourse.bass_utils import run_bass_kernel_spmd

    B, S, H = x.shape
    assert (B * S, H) == (NCORES * TPC, E)
    nc = _get_program()

    xf = np.ascontiguousarray(x.reshape(B * S, H))
    WqT = np.ascontiguousarray(Wq.T)
    WkT = np.ascontiguousarray(Wk.T)
    WvT = np.ascontiguousarray(Wv.T)
    WoT = np.ascontiguousarray(Wo.T)

    in_maps = []
    for i in range(NCORES):
        xT = np.ascontiguousarray(xf[i * TPC:(i + 1) * TPC, :].T)
        in_maps.append({"xT": xT, "WqT": WqT, "WkT": WkT,
                        "WvT": WvT, "WoT": WoT})

    import os
    trace = bool(int(os.environ.get("BASS_KERNEL_TRACE", "0")))
    res = run_bass_kernel_spmd(nc, in_maps, core_ids=list(range(NCORES)),
                               trace=trace)
    if trace:
        _cached["last_results"] = res
    parts = [res.results[i]["yT"].T for i in range(NCORES)]
    y = np.concatenate(parts, axis=0).reshape(B, S, H)
    return np.ascontiguousarray(y.astype(np.float32))
